# revision 13
# baseline (speedup 1.0000x reference)
"""TRN2 Bass kernel for CP-decoding line-sampling (nn_CPDecoding).

kernel(in_tensor [2097152,3] f32, line_coef [3,24,256] f32) -> [2097152] f32

Math per point n (reference semantics, align_corners grid_sample on R=256):
  pos_d = ((coord_d + 1) * 0.5) * 255          d=0,1,2 over (x,y,z) columns
  i0_d  = floor(pos_d); w_d = pos_d - i0_d
  f_d   = T_d[:, i0] + w_d * (T_d[:, i0+1] - T_d[:, i0])   (T_d = line_coef[2-d])
  out_n = sum_c f_0[c] * f_1[c] * f_2[c]

Strategy: data-parallel over points across 8 NeuronCores. The SWDGE gather
queues are the bottleneck (~15 GB/s per queue, 4 queues), so the kernel
minimizes gather descriptors per point:

  * y,z dims:  ONE 512B descriptor per point into a host-built 2D patch
    table. Product of two linear interps == bilinear interp of pairwise
    products, so unit (iy, s=iz>>1) stores lcY[c,iy+k] * lcZ[c,2s+j] for
    k<2, j<3 (the 3 z-columns make the unit parity-free; idx = iy*128+s
    fits int16). On-chip: 6 hat-bilinear weights per point.
  * x dim:     ONE 256B descriptor per point into a (base ++ delta) fp16
    pair table.

Gather indices are computed in place in the SWDGE wrapped layout from a
host-pre-arranged copy of the coordinates (one f32 per gather-index entry,
replicated into the two 16-partition groups of the consuming queue) — no
DRAM index bounce (the v1 kernel's bounce shredded into 6.3M 2-byte DMA
packets and cost 16ms).
"""

import sys

try:
    import concourse.bass  # noqa: F401
except Exception:
    sys.path.insert(0, "/opt/trn_rl_repo")

import numpy as np

import concourse.bacc as bacc
import concourse.bass as bass
import concourse.mybir as mybir
import concourse.tile as tile

F32 = mybir.dt.float32
F16 = mybir.dt.float16
I16 = mybir.dt.int16
COPY = mybir.ActivationFunctionType.Copy
RELU = mybir.ActivationFunctionType.Relu
ABS = mybir.ActivationFunctionType.Abs
ALU = mybir.AluOpType

N_TOTAL = 2097152
N_CORES = 8
N_PER_CORE = N_TOTAL // N_CORES
R = 256
C = 24
XES = 128        # x-table elem fp16 count (256B: 24 base, 24 delta, pad)
YZES = 256       # yz-table elem fp16 count (512B: 6 corners x 24, pad)
NT = 8192        # points per tile
GCHUNK = 1024    # idxs per gather instruction (SWDGE ring limit)
NQ = 4           # SWDGE queues

TILES = N_PER_CORE // NT
NCH = NT // 128          # point-chunks per partition per tile
GSUB = NT // GCHUNK      # sub-gathers per lookup-kind per tile
GNCH = GCHUNK // 128
GCOLS = GCHUNK // 16     # wrapped idx columns per sub-gather
QCOLS = 6 * GCOLS        # band cols: [x qn, x qn+4, y qn, y qn+4, z qn, z qn+4]


def build_xtab(line_coef: np.ndarray) -> np.ndarray:
    """x pair table [256, 128] fp16: row r = lc[2][:, r] ++ delta ++ pad."""
    X = np.ascontiguousarray(line_coef[2], dtype=np.float32)  # [24, 256]
    pt = np.zeros((R, XES), np.float16)
    pt[:, 0:C] = X.T
    pt[0 : R - 1, C : 2 * C] = (X[:, 1:R] - X[:, 0 : R - 1]).T
    return pt


def build_yztab(line_coef: np.ndarray) -> np.ndarray:
    """yz patch table [32768, 256] fp16.

    Unit u = iy*128 + s holds els (k*3 + j)*24 + c =
      lc[1][c, min(iy+k, 255)] * lc[0][c, min(2s+j, 255)],  k<2, j<3.
    """
    A = np.ascontiguousarray(line_coef[1], np.float32).T   # [256, C] (y)
    B = np.ascontiguousarray(line_coef[0], np.float32).T   # [256, C] (z)
    iy = np.minimum(np.arange(R)[:, None] + np.array([0, 1]), R - 1)
    zc = np.minimum(2 * np.arange(128)[:, None] + np.array([0, 1, 2]), R - 1)
    P = (A[iy][:, None, :, None, :] * B[zc][None, :, None, :, :])
    P = P.reshape(R * 128, 6 * C).astype(np.float16)
    tab = np.zeros((R * 128, YZES), np.float16)
    tab[:, 0 : 6 * C] = P
    return tab


def _slot_perm():
    i = np.arange(NT)
    return (((i // 16) % 8) * 16 + (i % 16)) * NCH + i // 128


def build_cwrap(shard: np.ndarray) -> np.ndarray:
    """[N_PER_CORE, 3] f32 -> [TILES*128, QCOLS] f32 wrapped gather coords.

    Band of queue qn (partitions 32qn..32qn+31, content duplicated in its
    two 16-partition groups): col regions cc hold coordinate column
    [0,0,1,1,2,2][cc] of the points of sub-gather chunk k = qn + 4*(cc%2),
    wrapped as entry l = col*16 + lane for tile slot i = k*GCHUNK + l.
    Slot i = ch*128 + h*16 + q handles point (h*16+q)*NCH + ch (gather
    output: partition i%128, free chunk i//128 -> block layout).
    """
    S = np.ascontiguousarray(shard, dtype=np.float32).reshape(TILES, NT, 3)
    P = S[:, _slot_perm(), :]                      # [TILES, slot, 3]
    cw = np.zeros((TILES, 128, QCOLS), np.float32)
    for qn in range(NQ):
        for cc in range(6):
            d = cc // 2
            k = qn + NQ * (cc % 2)
            blk = P[:, k * GCHUNK : (k + 1) * GCHUNK, d]       # [TILES, 1024]
            wq = blk.reshape(TILES, GCOLS, 16).transpose(0, 2, 1)
            cs = cc * GCOLS
            cw[:, 32 * qn : 32 * qn + 16, cs : cs + GCOLS] = wq
            cw[:, 32 * qn + 16 : 32 * qn + 32, cs : cs + GCOLS] = wq
    return cw.reshape(TILES * 128, QCOLS)


def build_kernel(n_per_core: int = N_PER_CORE, nt: int = NT, bufs: int = 2):
    assert n_per_core % nt == 0 and nt % 2048 == 0
    nch = NCH
    tiles = TILES

    nc = bacc.Bacc("TRN2", target_bir_lowering=False, num_swdge_queues=NQ)
    coords = nc.dram_tensor("coords", [n_per_core, 3], F32, kind="ExternalInput")
    cwrap = nc.dram_tensor("cwrap", [tiles * 128, QCOLS], F32, kind="ExternalInput")
    xtab = nc.dram_tensor("xtab", [R, XES], F16, kind="ExternalInput")
    yztab = nc.dram_tensor("yztab", [R * 128, YZES], F16, kind="ExternalInput")
    out = nc.dram_tensor("out", [n_per_core], F32, kind="ExternalOutput")

    with tile.TileContext(nc) as tc:
        with (
            tc.tile_pool(name="sb", bufs=bufs) as pool,
            tc.tile_pool(name="gt", bufs=bufs) as gpool,
        ):
            for t in range(tiles):
                # ---- wrapped idx path ----
                cwt = pool.tile([128, QCOLS], F32, tag="cwt")
                nc.sync.dma_start(cwt[:, :], cwrap.ap()[t * 128 : (t + 1) * 128, :])
                posw = pool.tile([128, QCOLS], F32, tag="posw")
                nc.scalar.activation(posw[:, :], cwt[:, :], COPY, bias=0.5, scale=0.5)
                nc.scalar.activation(posw[:, :], posw[:, :], COPY, bias=0.0, scale=255.0)
                rw16 = pool.tile([128, QCOLS], I16, tag="rw16")
                nc.vector.tensor_copy(rw16[:, :], posw[:, :])
                rwf = pool.tile([128, QCOLS], F32, tag="rwf")
                nc.vector.tensor_copy(rwf[:, :], rw16[:, :])
                gw = pool.tile([128, QCOLS], F32, tag="gw")
                nc.vector.tensor_tensor(
                    out=gw[:, :], in0=rwf[:, :], in1=posw[:, :], op=ALU.is_gt)
                i0w = pool.tile([128, QCOLS], F32, tag="i0w")
                nc.vector.tensor_tensor(
                    out=i0w[:, :], in0=rwf[:, :], in1=gw[:, :], op=ALU.subtract)
                # x idx: cols [0, 128)
                idxx = pool.tile([128, 2 * GCOLS], I16, tag="idxx")
                nc.vector.tensor_copy(idxx[:, :], i0w[:, 0 : 2 * GCOLS])
                # z anchor s = floor(pos_z * 0.5): halved-pos floor chain
                hz = pool.tile([128, 2 * GCOLS], F32, tag="hz")
                nc.scalar.activation(
                    hz[:, :], posw[:, 4 * GCOLS : 6 * GCOLS], COPY,
                    bias=0.0, scale=0.5)
                hr16 = pool.tile([128, 2 * GCOLS], I16, tag="hr16")
                nc.vector.tensor_copy(hr16[:, :], hz[:, :])
                hrf = pool.tile([128, 2 * GCOLS], F32, tag="hrf")
                nc.vector.tensor_copy(hrf[:, :], hr16[:, :])
                hgw = pool.tile([128, 2 * GCOLS], F32, tag="hgw")
                nc.vector.tensor_tensor(
                    out=hgw[:, :], in0=hrf[:, :], in1=hz[:, :], op=ALU.is_gt)
                sfl = pool.tile([128, 2 * GCOLS], F32, tag="sfl")
                nc.vector.tensor_tensor(
                    out=sfl[:, :], in0=hrf[:, :], in1=hgw[:, :], op=ALU.subtract)
                # yz idx = iy*128 + s
                t1 = pool.tile([128, 2 * GCOLS], F32, tag="t1")
                nc.scalar.activation(
                    t1[:, :], i0w[:, 2 * GCOLS : 4 * GCOLS], COPY,
                    bias=0.0, scale=128.0)
                nc.vector.tensor_tensor(
                    out=t1[:, :], in0=t1[:, :], in1=sfl[:, :], op=ALU.add)
                idxyz = pool.tile([128, 2 * GCOLS], I16, tag="idxyz")
                nc.vector.tensor_copy(idxyz[:, :], t1[:, :])

                # ---- gathers ----
                gx = gpool.tile([128, nch, XES], F16, tag="gx")
                gyz = gpool.tile([128, nch, YZES], F16, tag="gyz")
                for k in range(GSUB):
                    qn = k % NQ
                    cs = (k // NQ) * GCOLS
                    nc.gpsimd.dma_gather(
                        gx[:, k * GNCH : (k + 1) * GNCH, :], xtab.ap(),
                        idxx[:, cs : cs + GCOLS],
                        num_idxs=GCHUNK, num_idxs_reg=GCHUNK, elem_size=XES,
                        queue_num=qn)
                    nc.gpsimd.dma_gather(
                        gyz[:, k * GNCH : (k + 1) * GNCH, :], yztab.ap(),
                        idxyz[:, cs : cs + GCOLS],
                        num_idxs=GCHUNK, num_idxs_reg=GCHUNK, elem_size=YZES,
                        queue_num=qn)

                # ---- block path: per-point weights ----
                cb = pool.tile([128, nch * 3], F32, tag="cb")
                nc.sync.dma_start(
                    cb[:, :],
                    coords.ap()[t * nt : (t + 1) * nt, :]
                    .rearrange("(p j) c -> p (j c)", p=128))
                posb = pool.tile([128, nch * 3], F32, tag="posb")
                nc.scalar.activation(posb[:, :], cb[:, :], COPY, bias=0.5, scale=0.5)
                nc.scalar.activation(posb[:, :], posb[:, :], COPY, bias=0.0, scale=255.0)
                r16 = pool.tile([128, nch * 3], I16, tag="r16")
                nc.vector.tensor_copy(r16[:, :], posb[:, :])
                rf = pool.tile([128, nch * 3], F32, tag="rf")
                nc.vector.tensor_copy(rf[:, :], r16[:, :])
                g = pool.tile([128, nch * 3], F32, tag="g")
                nc.vector.tensor_tensor(
                    out=g[:, :], in0=rf[:, :], in1=posb[:, :], op=ALU.is_gt)
                i0f = pool.tile([128, nch * 3], F32, tag="i0f")
                nc.vector.tensor_tensor(
                    out=i0f[:, :], in0=rf[:, :], in1=g[:, :], op=ALU.subtract)
                wf = pool.tile([128, nch * 3], F32, tag="wf")
                nc.vector.tensor_tensor(
                    out=wf[:, :], in0=posb[:, :], in1=i0f[:, :], op=ALU.subtract)
                wv = wf[:, :].rearrange("p (j c) -> p c j", c=3)
                posv = posb[:, :].rearrange("p (j c) -> p c j", c=3)

                # z-anchor (block side, identical arithmetic to wrapped side)
                hzb = pool.tile([128, nch], F32, tag="hzb")
                nc.scalar.activation(
                    hzb[:, :],
                    posv[:, 2:3, :].rearrange("p o j -> p (o j)"),
                    COPY, bias=0.0, scale=0.5)
                hb16 = pool.tile([128, nch], I16, tag="hb16")
                nc.vector.tensor_copy(hb16[:, :], hzb[:, :])
                hbf = pool.tile([128, nch], F32, tag="hbf")
                nc.vector.tensor_copy(hbf[:, :], hb16[:, :])
                hbg = pool.tile([128, nch], F32, tag="hbg")
                nc.vector.tensor_tensor(
                    out=hbg[:, :], in0=hbf[:, :], in1=hzb[:, :], op=ALU.is_gt)
                sb = pool.tile([128, nch], F32, tag="sb")
                nc.vector.tensor_tensor(
                    out=sb[:, :], in0=hbf[:, :], in1=hbg[:, :], op=ALU.subtract)
                # u = pos_z - 2*s  in [0, 2)
                u2 = pool.tile([128, nch], F32, tag="u2")
                nc.scalar.activation(u2[:, :], sb[:, :], COPY, bias=0.0, scale=2.0)
                uu = pool.tile([128, nch], F32, tag="uu")
                nc.vector.tensor_tensor(
                    out=uu[:, :],
                    in0=posv[:, 2:3, :].rearrange("p o j -> p (o j)"),
                    in1=u2[:, :], op=ALU.subtract)
                # hat weights wz0 = relu(1-u), wz1 = 1-|u-1|, wz2 = relu(u-1)
                # (affine parts via Copy: float bias lowers to an immediate
                # only for Copy; Relu/Abs would need a registered const AP)
                d1 = pool.tile([128, nch], F32, tag="d1")
                nc.scalar.activation(d1[:, :], uu[:, :], COPY, bias=-1.0, scale=1.0)
                m1 = pool.tile([128, nch], F32, tag="m1")
                nc.scalar.activation(m1[:, :], uu[:, :], COPY, bias=1.0, scale=-1.0)
                wz0 = pool.tile([128, nch], F32, tag="wz0")
                nc.scalar.activation(wz0[:, :], m1[:, :], RELU, bias=0.0, scale=1.0)
                wza = pool.tile([128, nch], F32, tag="wza")
                nc.scalar.activation(wza[:, :], d1[:, :], ABS, bias=0.0, scale=1.0)
                wz1 = pool.tile([128, nch], F32, tag="wz1")
                nc.scalar.activation(wz1[:, :], wza[:, :], COPY, bias=1.0, scale=-1.0)
                wz2 = pool.tile([128, nch], F32, tag="wz2")
                nc.scalar.activation(wz2[:, :], d1[:, :], RELU, bias=0.0, scale=1.0)
                # wy0 = 1 - wy, wy1 = wy
                wy1 = wv[:, 1:2, :].rearrange("p o j -> p (o j)")
                wy0 = pool.tile([128, nch], F32, tag="wy0")
                nc.scalar.activation(wy0[:, :], wy1, COPY, bias=1.0, scale=-1.0)
                # 7 fp16 weight rows: 6 corner products (k*3+j) + wx
                wcf = pool.tile([128, 7, nch], F32, tag="wcf")
                for kk, wyk in ((0, wy0[:, :]), (1, None)):
                    for j, wzj in enumerate((wz0, wz1, wz2)):
                        dst = wcf[:, kk * 3 + j : kk * 3 + j + 1, :] \
                            .rearrange("p o j -> p (o j)")
                        nc.vector.tensor_tensor(
                            out=dst, in0=(wyk if wyk is not None else wy1),
                            in1=wzj[:, :], op=ALU.mult)
                nc.vector.tensor_copy(
                    wcf[:, 6:7, :].rearrange("p o j -> p (o j)"),
                    wv[:, 0:1, :].rearrange("p o j -> p (o j)"))
                wc = pool.tile([128, 7, nch], F16, tag="wc")
                nc.vector.tensor_copy(
                    wc[:, :, :].rearrange("p a j -> p (a j)"),
                    wcf[:, :, :].rearrange("p a j -> p (a j)"))

                # ---- hat-bilinear accumulate + x interp + product + reduce ----
                facc = pool.tile([128, nch, C], F16, tag="facc")
                tsc = pool.tile([128, nch, C], F16, tag="tsc")
                for m in range(6):
                    wb = wc[:, m : m + 1, :].rearrange("p o j -> p (o j)") \
                        .unsqueeze(2).broadcast_to([128, nch, C])
                    if m == 0:
                        nc.vector.tensor_tensor(
                            out=facc[:, :, :],
                            in0=gyz[:, :, m * C : (m + 1) * C],
                            in1=wb, op=ALU.mult)
                    else:
                        nc.vector.tensor_tensor(
                            out=tsc[:, :, :],
                            in0=gyz[:, :, m * C : (m + 1) * C],
                            in1=wb, op=ALU.mult)
                        nc.vector.tensor_tensor(
                            out=facc[:, :, :], in0=facc[:, :, :],
                            in1=tsc[:, :, :], op=ALU.add)
                wxb = wc[:, 6:7, :].rearrange("p o j -> p (o j)") \
                    .unsqueeze(2).broadcast_to([128, nch, C])
                fx = pool.tile([128, nch, C], F16, tag="fx")
                nc.vector.tensor_tensor(
                    out=fx[:, :, :], in0=gx[:, :, C : 2 * C], in1=wxb,
                    op=ALU.mult)
                nc.vector.tensor_tensor(
                    out=fx[:, :, :], in0=fx[:, :, :], in1=gx[:, :, 0:C],
                    op=ALU.add)
                nc.vector.tensor_tensor(
                    out=facc[:, :, :], in0=facc[:, :, :], in1=fx[:, :, :],
                    op=ALU.mult)
                res = pool.tile([128, nch], F32, tag="res")
                nc.vector.tensor_reduce(
                    out=res[:, :], in_=facc[:, :, :],
                    axis=mybir.AxisListType.X, op=ALU.add)
                nc.sync.dma_start(
                    out.ap()[t * nt : (t + 1) * nt].rearrange("(p j) -> p j", p=128),
                    res[:, :])
    nc.compile()
    return nc


_NC_CACHE = {}


def _get_nc():
    key = (N_PER_CORE, NT, GCHUNK)
    if key not in _NC_CACHE:
        _NC_CACHE[key] = build_kernel()
    return _NC_CACHE[key]


def run(in_tensor: np.ndarray, line_coef: np.ndarray, trace: bool = False):
    """Returns (out [N_TOTAL] f32, BassKernelResults)."""
    from concourse.bass_utils import run_bass_kernel_spmd

    in_tensor = np.ascontiguousarray(in_tensor, dtype=np.float32)
    assert in_tensor.shape == (N_TOTAL, 3)
    line_coef = np.asarray(line_coef)
    xtab = build_xtab(line_coef)
    yztab = build_yztab(line_coef)
    nc = _get_nc()
    shards = in_tensor.reshape(N_CORES, N_PER_CORE, 3)
    in_maps = [
        {"coords": shards[i], "cwrap": build_cwrap(shards[i]),
         "xtab": xtab, "yztab": yztab}
        for i in range(N_CORES)
    ]
    res = run_bass_kernel_spmd(nc, in_maps, core_ids=list(range(N_CORES)),
                               trace=trace)
    out = np.concatenate([np.asarray(r["out"]) for r in res.results])
    return out, res


def kernel(in_tensor: np.ndarray, line_coef: np.ndarray) -> np.ndarray:
    out, _ = run(np.asarray(in_tensor), np.asarray(line_coef))
    return out
